# revision 7
# baseline (speedup 1.0000x reference)
"""Trainium2 Bass kernel for nn_MiniDecoder: 8-layer decoder transformer.

Sharding: sequence-parallel over token blocks across 8 cores.
  T=2048 per batch = 16 blocks of 128; core c owns blocks {c, 15-c} of each
  of the B=2 batches (512 token rows per core) -- balanced causal work.
  All weights replicated (streamed bf16 from HBM); per-layer K/V exchanged
  with a single AllGather. LM head is row-parallel (zero comm).

On-chip layout: activations feature-major [128 part = feat%128, fchunk, tok].
  Attention computes S^T = K^T Q per head (K=64 matmuls at partition bases
  0/64), exp on ACT (scale=1/8 folded), causal handled by per-core mask
  tensors (uniform SPMD program), P*V via a 2-segment stationary AP that
  appends a ones-block -> softmax denominator lands in PSUM rows 64:127.
  RMSNorm via ones-matmul partition reduction; ln weights folded into the
  following weight matrices host-side; rsqrt = exp(-0.5*ln(ms+eps)).
"""
import os
import sys

sys.path.insert(0, os.path.dirname(os.path.abspath(__file__)))

import numpy as np
import ml_dtypes

import concourse.bass as bass
import concourse.bacc as bacc
import concourse.mybir as mybir
import concourse.tile as tile
from concourse.bass_utils import run_bass_kernel_spmd
from concourse.masks import make_identity

try:
    import ntff_hook  # noqa: F401  (enables trace=True HW timing under axon)
except Exception:
    pass

B, T, D, H, HD, L, DI, V = 2, 2048, 1024, 16, 64, 8, 2048, 32000
NCORES = 8
NB = 16          # 128-token blocks per batch
TPC = 512        # token rows per core (B * 2 blocks * 128)
EPS = 1e-6
KSZ = D * 256    # per-batch k region elems in ag buffer
VSZ = 256 * D    # per-batch v region elems
AGE = 2 * KSZ + 2 * VSZ  # per-rank allgather elems (bf16)
NVT = 63         # head vocab tiles: 62 x 512 + 1 x 256

bf16 = mybir.dt.bfloat16
f32 = mybir.dt.float32
i32 = mybir.dt.int32
AF = mybir.ActivationFunctionType

_CACHE = {}


def _owner(g):
    """global block g -> (rank, slot)"""
    return (g, 0) if g < 8 else (15 - g, 1)


def build_nc():
    nc = bacc.Bacc("TRN2", target_bir_lowering=False, debug=False, num_devices=NCORES)

    emb32 = nc.dram_tensor("emb32", [V, D], f32, kind="ExternalInput")
    embT = nc.dram_tensor("embT", [D, V], bf16, kind="ExternalInput")
    wqkT = nc.dram_tensor("wqkT", [L, D, 2 * D], bf16, kind="ExternalInput")
    wvT = nc.dram_tensor("wvT", [L, D, D], bf16, kind="ExternalInput")
    woT = nc.dram_tensor("woT", [L, D, D], bf16, kind="ExternalInput")
    wgT = nc.dram_tensor("wgT", [L, D, DI], bf16, kind="ExternalInput")
    wuT = nc.dram_tensor("wuT", [L, D, DI], bf16, kind="ExternalInput")
    wdT = nc.dram_tensor("wdT", [L, DI, D], bf16, kind="ExternalInput")
    idxs = nc.dram_tensor("idxs", [TPC, 1], i32, kind="ExternalInput")
    masks = nc.dram_tensor("masks", [16, 128, 256], bf16, kind="ExternalInput")
    out = nc.dram_tensor("out", [TPC, V], f32, kind="ExternalOutput")

    ag_in = nc.dram_tensor("ag_in", [AGE], bf16)
    ag_out = nc.dram_tensor("ag_out", [NCORES * AGE], bf16, addr_space="Shared")

    def kreg(r, b):
        base = r * AGE + b * KSZ
        return ag_out.ap()[base : base + KSZ].rearrange("(f t) -> f t", t=256)

    def vreg(r, b):
        base = r * AGE + 2 * KSZ + b * VSZ
        return ag_out.ap()[base : base + VSZ].rearrange("(t f) -> t f", f=D)

    def kreg_in(b):
        return ag_in.ap()[b * KSZ : (b + 1) * KSZ].rearrange("(f t) -> f t", t=256)

    def vreg_in(b):
        base = 2 * KSZ + b * VSZ
        return ag_in.ap()[base : base + VSZ].rearrange("(t f) -> t f", f=D)

    with tile.TileContext(nc) as tc:
        with (
            tc.tile_pool(name="const", bufs=1) as const,
            tc.tile_pool(name="pers", bufs=1) as pers,
            tc.tile_pool(name="big", bufs=1) as big,
            tc.tile_pool(name="wp", bufs=3) as wp,
            tc.tile_pool(name="w512", bufs=3) as w512,
            tc.tile_pool(name="kv", bufs=2) as kv,
            tc.tile_pool(name="work", bufs=3) as work,
            tc.tile_pool(name="ework", bufs=4) as ework,
            tc.tile_pool(name="ps_s", bufs=2, space="PSUM") as ps_s,
            tc.tile_pool(name="ps_av", bufs=1, space="PSUM") as ps_av,
            tc.tile_pool(name="ps_mm", bufs=2, space="PSUM") as ps_mm,
        ):
            ident = const.tile([128, 128], f32)
            make_identity(nc, ident)
            ones = const.tile([128, 128], bf16)
            nc.vector.memset(ones[:], 1.0)
            eps_t = const.tile([128, 1], f32)
            nc.vector.memset(eps_t[:], EPS)
            masks_sb = const.tile([128, 16, 256], bf16)
            nc.sync.dma_start(masks_sb[:], masks.ap().rearrange("j p q -> p j q"))
            idx_sb = const.tile([128, 4], i32)
            nc.sync.dma_start(
                idx_sb[:], idxs.ap().rearrange("(t p) one -> p (t one)", p=128)
            )

            # persistent activations
            x_fm = pers.tile([128, 8, TPC], f32, name="x_fm")

            # ---- prologue: embedding gather + transpose to feature-major ----
            for t in range(4):
                g = work.tile([128, D], f32, tag="xg")
                nc.gpsimd.indirect_dma_start(
                    out=g[:],
                    out_offset=None,
                    in_=emb32.ap(),
                    in_offset=bass.IndirectOffsetOnAxis(ap=idx_sb[:, t : t + 1], axis=0),
                )
                for fc in range(8):
                    tp = ps_mm.tile([128, 128], f32, tag="mm", name="tp")
                    nc.tensor.transpose(tp[:], g[:, fc * 128 : (fc + 1) * 128], ident[:])
                    nc.vector.tensor_copy(x_fm[:, fc, t * 128 : (t + 1) * 128], tp[:])

            def rmsnorm_r(tag):
                """returns r [128, 512] f32 (per-token rsqrt, replicated on parts)"""
                x2 = big.tile([128, 8, TPC], bf16, tag="big16", name="x2_" + tag)
                nc.vector.tensor_mul(x2[:], x_fm[:], x_fm[:])
                sq = ps_mm.tile([128, TPC], f32, tag="mm", name="sq_" + tag)
                for kc in range(8):
                    nc.tensor.matmul(
                        sq[:], ones[:], x2[:, kc, :], start=(kc == 0), stop=(kc == 7)
                    )
                lnms = work.tile([128, TPC], f32, tag="r0", name="lnms_" + tag)
                nc.scalar.activation(lnms[:], sq[:], AF.Ln, bias=eps_t[:], scale=1.0 / D)
                r = work.tile([128, TPC], f32, tag="r1", name="r_" + tag)
                nc.scalar.activation(r[:], lnms[:], AF.Exp, scale=-0.5)
                return r

            def scaled_act(r, tag):
                h = pers.tile([128, 8, TPC], bf16, tag="hg", name="h_" + tag)
                nc.vector.tensor_mul(
                    h[:], x_fm[:], r[:, None, :].to_broadcast([128, 8, TPC])
                )
                return h

            # ============================ layers ============================
            for l in range(L):
                r1 = rmsnorm_r(f"n1_{l}")
                h = scaled_act(r1, f"{l}")

                wqk_v = wqkT.ap()[l].rearrange("(kc p) o -> p kc o", p=128)

                # ---- k projection (feats 1024:2048 of wqk) -> ag_in ----
                for fo in range(8, 16):
                    wt = wp.tile([128, 8, 128], bf16, tag="w128", name=f"wk_{l}_{fo}")
                    nc.sync.dma_start(wt[:], wqk_v[:, :, fo * 128 : (fo + 1) * 128])
                    pk = ps_mm.tile([128, TPC], f32, tag="mm", name=f"pk_{l}_{fo}")
                    for kc in range(8):
                        nc.tensor.matmul(
                            pk[:], wt[:, kc, :], h[:, kc, :],
                            start=(kc == 0), stop=(kc == 7),
                        )
                    ks = work.tile([128, TPC], bf16, tag="kst", name=f"ks_{l}_{fo}")
                    nc.scalar.copy(ks[:], pk[:])
                    fr = (fo - 8) * 128
                    for b in range(2):
                        nc.sync.dma_start(
                            kreg_in(b)[fr : fr + 128, :], ks[:, b * 256 : (b + 1) * 256]
                        )

                # ---- v projection (token-major) -> ag_in ----
                for vo in range(2):
                    wv_t = w512.tile([128, 8, 512], bf16, tag="w512", name=f"wv_{l}_{vo}")
                    nc.sync.dma_start(
                        wv_t[:],
                        wvT.ap()[l].rearrange("(kc p) o -> p kc o", p=128)[
                            :, :, vo * 512 : (vo + 1) * 512
                        ],
                    )
                    for t in range(4):
                        pv = ps_mm.tile([128, 512], f32, tag="mm", name=f"pv_{l}_{vo}_{t}")
                        for kc in range(8):
                            nc.tensor.matmul(
                                pv[:], h[:, kc, t * 128 : (t + 1) * 128],
                                wv_t[:, kc, :], start=(kc == 0), stop=(kc == 7),
                            )
                        vs = work.tile([128, 512], bf16, tag="vst", name=f"vs_{l}_{vo}_{t}")
                        nc.scalar.copy(vs[:], pv[:])
                        nc.sync.dma_start(
                            vreg_in(t // 2)[
                                (t % 2) * 128 : (t % 2) * 128 + 128,
                                vo * 512 : (vo + 1) * 512,
                            ],
                            vs[:],
                        )

                # ---- allgather K/V ----
                nc.gpsimd.collective_compute(
                    "AllGather",
                    mybir.AluOpType.bypass,
                    replica_groups=[list(range(NCORES))],
                    ins=[ag_in.ap()],
                    outs=[ag_out.ap()],
                )

                # ---- q projection (feats 0:1024), overlaps AG ----
                q_fm = pers.tile([128, 8, TPC], bf16, tag="q", name=f"q_{l}")
                for fo in range(8):
                    wt = wp.tile([128, 8, 128], bf16, tag="w128", name=f"wq_{l}_{fo}")
                    nc.sync.dma_start(wt[:], wqk_v[:, :, fo * 128 : (fo + 1) * 128])
                    pq = ps_mm.tile([128, TPC], f32, tag="mm", name=f"pq_{l}_{fo}")
                    for kc in range(8):
                        nc.tensor.matmul(
                            pq[:], wt[:, kc, :], h[:, kc, :],
                            start=(kc == 0), stop=(kc == 7),
                        )
                    nc.scalar.copy(q_fm[:, fo, :], pq[:])

                # ---- attention ----
                y_fm = pers.tile([128, 8, TPC], bf16, tag="y", name=f"y_{l}")
                for b in range(2):
                    for p in range(8):
                        k_pair = kv.tile([128, 2048], bf16, tag="kp", name=f"kp_{l}_{b}_{p}")
                        for rr in range(8):
                            src = kreg(rr, b)[p * 128 : (p + 1) * 128, :]
                            for s in range(2):
                                gblk = rr if s == 0 else 15 - rr
                                nc.sync.dma_start(
                                    k_pair[:, gblk * 128 : (gblk + 1) * 128],
                                    src[:, s * 128 : (s + 1) * 128],
                                )
                        # per head: 64 v-cols then 64 ones-cols (contiguous 128
                        # stationary block -> denominator in PSUM rows 64:127)
                        v_pair = kv.tile([128, 16, 2, 128], bf16, tag="vp", name=f"vp_{l}_{b}_{p}")
                        for g in range(16):
                            rr, s = _owner(g)
                            src = vreg(rr, b)[
                                s * 128 : (s + 1) * 128, p * 128 : (p + 1) * 128
                            ].rearrange("t (hh f) -> t hh f", hh=2)
                            nc.sync.dma_start(v_pair[:, g, :, 0:64], src)
                        nc.vector.memset(v_pair[:, :, :, 64:128], 1.0)

                        av0 = ps_av.tile([128, 256], f32, tag="av0", name=f"av0_{l}_{b}_{p}")
                        av1 = ps_av.tile([128, 256], f32, tag="av1", name=f"av1_{l}_{b}_{p}")
                        qA = q_fm[0:64, p, b * 256 : (b + 1) * 256]
                        qB = q_fm[64:128, p, b * 256 : (b + 1) * 256]
                        for j in range(16):
                            s0 = ps_s.tile([128, 256], f32, tag="s0", name=f"s0_{l}_{b}_{p}_{j}")
                            s1 = ps_s.tile([128, 256], f32, tag="s1", name=f"s1_{l}_{b}_{p}_{j}")
                            kc_sl = slice(j * 128, (j + 1) * 128)
                            nc.tensor.matmul(s0[:], k_pair[0:64, kc_sl], qA, start=True, stop=True)
                            nc.tensor.matmul(s1[:], k_pair[64:128, kc_sl], qB, start=True, stop=True)
                            e0 = ework.tile([128, 256], bf16, tag="e0", name=f"e0_{l}_{b}_{p}_{j}")
                            e1 = ework.tile([128, 256], bf16, tag="e1", name=f"e1_{l}_{b}_{p}_{j}")
                            nc.scalar.activation(e0[:], s0[:], AF.Exp, scale=0.125)
                            nc.scalar.activation(e1[:], s1[:], AF.Exp, scale=0.125)
                            nc.vector.tensor_mul(e0[:], e0[:], masks_sb[:, j, :])
                            nc.vector.tensor_mul(e1[:], e1[:], masks_sb[:, j, :])
                            nc.tensor.matmul(av0[:], v_pair[:, j, 0, :], e0[:], start=(j == 0), stop=(j == 15))
                            nc.tensor.matmul(av1[:], v_pair[:, j, 1, :], e1[:], start=(j == 0), stop=(j == 15))
                        for hh, av in ((0, av0), (1, av1)):
                            rec = ework.tile([64, 256], f32, tag="rec", name=f"rec_{l}_{b}_{p}_{hh}")
                            nc.vector.reciprocal(rec[:], av[64:128, :])
                            nc.vector.tensor_mul(
                                y_fm[hh * 64 : hh * 64 + 64, p, b * 256 : (b + 1) * 256],
                                av[0:64, :], rec[:],
                            )

                # ---- o projection + residual ----
                wo_v = woT.ap()[l].rearrange("(kc p) o -> p kc o", p=128)
                for fo in range(8):
                    wt = wp.tile([128, 8, 128], bf16, tag="w128", name=f"wo_{l}_{fo}")
                    nc.sync.dma_start(wt[:], wo_v[:, :, fo * 128 : (fo + 1) * 128])
                    po = ps_mm.tile([128, TPC], f32, tag="mm", name=f"po_{l}_{fo}")
                    for kc in range(8):
                        nc.tensor.matmul(
                            po[:], wt[:, kc, :], y_fm[:, kc, :],
                            start=(kc == 0), stop=(kc == 7),
                        )
                    nc.vector.tensor_add(x_fm[:, fo, :], x_fm[:, fo, :], po[:])

                # ---- mlp ----
                r2 = rmsnorm_r(f"n2_{l}")
                gct = scaled_act(r2, f"g{l}")
                gu = big.tile([128, 16, TPC], bf16, tag="big16", name=f"gu_{l}")
                wg_v = wgT.ap()[l].rearrange("(kc p) o -> p kc o", p=128)
                wu_v = wuT.ap()[l].rearrange("(kc p) o -> p kc o", p=128)
                for fo in range(16):
                    wtg = wp.tile([128, 8, 128], bf16, tag="w128", name=f"wg_{l}_{fo}")
                    nc.sync.dma_start(wtg[:], wg_v[:, :, fo * 128 : (fo + 1) * 128])
                    pg = ps_mm.tile([128, TPC], f32, tag="mm", name=f"pg_{l}_{fo}")
                    for kc in range(8):
                        nc.tensor.matmul(
                            pg[:], wtg[:, kc, :], gct[:, kc, :],
                            start=(kc == 0), stop=(kc == 7),
                        )
                    gs = work.tile([128, TPC], bf16, tag="gs", name=f"gs_{l}_{fo}")
                    nc.scalar.activation(gs[:], pg[:], AF.Silu)
                    wtu = wp.tile([128, 8, 128], bf16, tag="w128", name=f"wu_{l}_{fo}")
                    nc.sync.dma_start(wtu[:], wu_v[:, :, fo * 128 : (fo + 1) * 128])
                    pu = ps_mm.tile([128, TPC], f32, tag="mm", name=f"pu_{l}_{fo}")
                    for kc in range(8):
                        nc.tensor.matmul(
                            pu[:], wtu[:, kc, :], gct[:, kc, :],
                            start=(kc == 0), stop=(kc == 7),
                        )
                    nc.vector.tensor_mul(gu[:, fo, :], gs[:], pu[:])

                wd_v = wdT.ap()[l].rearrange("(kc p) o -> p kc o", p=128)
                for fo in range(8):
                    wtd = wp.tile([128, 16, 128], bf16, tag="w256", name=f"wd_{l}_{fo}")
                    nc.sync.dma_start(wtd[:], wd_v[:, :, fo * 128 : (fo + 1) * 128])
                    pd = ps_mm.tile([128, TPC], f32, tag="mm", name=f"pd_{l}_{fo}")
                    for kc in range(16):
                        nc.tensor.matmul(
                            pd[:], wtd[:, kc, :], gu[:, kc, :],
                            start=(kc == 0), stop=(kc == 15),
                        )
                    nc.vector.tensor_add(x_fm[:, fo, :], x_fm[:, fo, :], pd[:])

            # ============================ lm head ============================
            rf = rmsnorm_r("nf")
            xf = scaled_act(rf, "f")
            embT_v = embT.ap().rearrange("(kc p) v -> p kc v", p=128)
            for vt in range(NVT):
                n = 512 if vt < 62 else 256
                et = w512.tile([128, 8, 512], bf16, tag="w512", name=f"et_{vt}")
                nc.sync.dma_start(et[:, :, :n], embT_v[:, :, vt * 512 : vt * 512 + n])
                for t in range(4):
                    ph = ps_mm.tile([128, 512], f32, tag="mm", name=f"ph_{vt}_{t}")
                    for kc in range(8):
                        nc.tensor.matmul(
                            ph[:, :n], xf[:, kc, t * 128 : (t + 1) * 128],
                            et[:, kc, :n], start=(kc == 0), stop=(kc == 7),
                        )
                    ot = work.tile([128, 512], f32, tag="oev", name=f"ot_{vt}_{t}")
                    nc.vector.tensor_copy(ot[:, :n], ph[:, :n])
                    nc.sync.dma_start(
                        out.ap()[t * 128 : (t + 1) * 128, vt * 512 : vt * 512 + n],
                        ot[:, :n],
                    )

    nc.compile()
    return nc


def _prep_host(idx, emb, ln1_w, qkv_w, o_w, ln2_w, gate_w, up_w, down_w, lnf_w):
    bf = ml_dtypes.bfloat16
    f = np.float32
    idx = np.asarray(idx)
    emb = np.asarray(emb, dtype=f)
    ln1 = np.asarray(ln1_w, dtype=f)
    ln2 = np.asarray(ln2_w, dtype=f)
    lnf = np.asarray(lnf_w, dtype=f)
    qkv = np.asarray(qkv_w, dtype=f)

    wqkT = np.ascontiguousarray(
        np.transpose(qkv[:, : 2 * D, :] * ln1[:, None, :], (0, 2, 1))
    ).astype(bf)
    wvT = np.ascontiguousarray(
        np.transpose(qkv[:, 2 * D :, :] * ln1[:, None, :], (0, 2, 1))
    ).astype(bf)
    woT = np.ascontiguousarray(np.transpose(np.asarray(o_w, f), (0, 2, 1))).astype(bf)
    wgT = np.ascontiguousarray(
        np.transpose(np.asarray(gate_w, f) * ln2[:, None, :], (0, 2, 1))
    ).astype(bf)
    wuT = np.ascontiguousarray(
        np.transpose(np.asarray(up_w, f) * ln2[:, None, :], (0, 2, 1))
    ).astype(bf)
    wdT = np.ascontiguousarray(np.transpose(np.asarray(down_w, f), (0, 2, 1))).astype(bf)
    embT = np.ascontiguousarray((emb * lnf[None, :]).T).astype(bf)
    emb32 = np.ascontiguousarray(emb)

    shared = dict(emb32=emb32, embT=embT, wqkT=wqkT, wvT=wvT, woT=woT,
                  wgT=wgT, wuT=wuT, wdT=wdT)

    # S^T layout: rows = key positions, cols = query positions within the
    # block -> causal (key <= query) is the *upper* triangle incl. diagonal.
    triu = np.triu(np.ones((128, 128), f))
    in_maps = []
    for c in range(NCORES):
        blocks = [c, 15 - c]
        idx_c = np.concatenate(
            [idx[b, g * 128 : (g + 1) * 128] for b in range(B) for g in blocks]
        ).astype(np.int32)[:, None]
        m = np.zeros((16, 128, 256), f)
        for j in range(16):
            for qi, qb in enumerate(blocks):
                col = qi * 128
                if j < qb:
                    m[j, :, col : col + 128] = 1.0
                elif j == qb:
                    m[j, :, col : col + 128] = triu
        in_maps.append(
            dict(shared, idxs=idx_c, masks=m.astype(ml_dtypes.bfloat16))
        )
    return in_maps


LAST_RESULT = None


def kernel(idx, emb, ln1_w, qkv_w, o_w, ln2_w, gate_w, up_w, down_w, lnf_w):
    global LAST_RESULT
    if "nc" not in _CACHE:
        _CACHE["nc"] = build_nc()
    nc = _CACHE["nc"]
    in_maps = _prep_host(idx, emb, ln1_w, qkv_w, o_w, ln2_w, gate_w, up_w, down_w, lnf_w)
    res = run_bass_kernel_spmd(nc, in_maps, core_ids=list(range(NCORES)))
    LAST_RESULT = res

    full = np.empty((B, T, V), np.float32)
    for c in range(NCORES):
        o = res.results[c]["out"]
        blocks = [c, 15 - c]
        i = 0
        for b in range(B):
            for g in blocks:
                full[b, g * 128 : (g + 1) * 128, :] = o[i * 128 : (i + 1) * 128, :]
                i += 1
    return full
